# revision 58
# baseline (speedup 1.0000x reference)
"""Trainium2 Bass kernel for nn_CustomNetwork_31585189494999 (gnn_message_passing).

Reference computation (per record b of B=65536, per non-root node n of 256):
  xg = x[:, parent_idx]                      # gather [B, 256, 8]
  h1 = einsum(xg, W1) + b1                   # per-node Linear(8->8)
  a1 = selu(batchnorm(h1))                   # BN over batch, per (node, ch)
  h2 = einsum(a1, W2) + b2                   # per-node Linear(8->8)
  a2 = selu(batchnorm(h2))
  out = einsum(a2, W3) + b3  ; sigmoid on last node only

Device strategy (8 NeuronCores, batch-sharded 8192 records/core).
Host->device bytes are the dominant cost, so per core we ship ONLY:
  * xT [320, Bs] f16 -- the core's x shard, feature-major (5.25 MB);
  * wsm [8, 832] f16 -- dense W1/W2/W3 rows + parent_idx for this core's
    1/8 channel shard (13 KB); the scattered A1 and block-diagonal W2/W3
    are BUILT on device (iota/affine masks) and exchanged by AllGather;
  * gbh [128, 64] f16 + b3t [128, 4] f32 -- gamma/beta group layout
    (f16-exact) and the folded b3 (18 KB).
On device:
  * The gather fuses into the A-form A1[c, n*8+h] = sum_d [pidx==c] W1;
    layer 1 is 3 accumulating K<=128 matmuls straight from xT chunks.
  * BN1 statistics: C = x^T x gram (b-major tiles made on-device via
    XBAR dma transpose of xT), AllReduce of the projected [1,2048] rows:
    E[h1^2] = diag(A1^T (C/B) A1); mean1 via bn_stats feature means
    projected through A1. b1/b2 drop (BN kills them).
  * SELU branch-free: selu(z)/LAM = relu(z) + AL*e^{min(z,0)} - AL.
    Constants fold into W2/W3/b3 on host. Engine paths per unit,
    mix-tunable: EXP4 (2 scalar + gpsimd add) / COMB4 (scalar exp +
    vector combine).
  * Phase B computes h1->y1 (spilled f16, g-major) and h2 (PSUM only,
    bn_stats); BN2 stats AllReduced; phase C reloads y1, recomputes h2
    with one matmul, applies BN2+SELU, block-diag W3 -> out [256, Bs].
  * Sigmoid of the last node runs on host over the final output column.
"""

import math
import os
import sys

for _p in ("/opt/trn_rl_repo",):
    if _p not in sys.path:
        sys.path.insert(0, _p)

import numpy as np

import concourse.bass as bass
import concourse.mybir as mybir
import concourse.tile as tile
from concourse import bacc
from concourse.bass_utils import run_bass_kernel_spmd

F16 = mybir.dt.float16
F32 = mybir.dt.float32

NCORES = 8
NTOT = 320
NSUB = 256
DD = 8
HH = 8
CH = NSUB * HH            # 2048 channels
G = 16                    # channel groups of 128
CHUNK = 1024              # batch tile width
EPS = 1e-5
LAM = 1.0507009873554805
AL = 1.6732632423543772

# Weight handling: each core BUILDS the 1/8 channel-shard it owns
# (256 channels = 32 nodes) of the pack (3 A1 k-tiles | W2 blockdiag |
# W3 blockdiag) from dense W1/W2/W3 rows, so only ~13 KB of weights
# upload per core. The shards are exchanged via an AllGather in the
# [16, 8704] -> [128, 8704] shape (proven stable; the [128w, .]-per-rank
# shape intermittently corrupted its tail rows), with DRAM rearrange
# bounces to re-pack the built [128, 1088] tile into 16 rows and back.
KT = [(0, 128), (128, 128), (256, 64)]
CSH = CH // NCORES        # 256 channels built per core
W2L = 3 * CSH             # wloc col offsets: a1 k-tiles | w2 | w3
W3L = W2L + CSH           # 1024
WLOC = W3L + 64           # 1088
WAGC = NCORES * WLOC      # 8704 AllGather payload cols
# small-weights input columns: pidx | w1 | w2 | w3
SM_W1 = CSH
SM_W2 = 2 * CSH
SM_W3 = 3 * CSH
SMCOLS = 3 * CSH + 64     # 832

# engine-mix knobs, per 16 units: MG4 units take EXP4G (scalar exp+relu,
# gpsimd min+add), the next ME4 take EXP4 (gpsimd add, vector min), the
# rest COMB4 (scalar exp + vector combine).
ME4_1 = int(os.environ.get("KME4_1", "12"))  # phase-B EXP4 units per 16
MG4_1 = int(os.environ.get("KMG4_1", "0"))   # phase-B EXP4G units per 16
ME4_2 = int(os.environ.get("KME4_2", "2"))   # phase-C EXP4 units per 16
MG4_2 = int(os.environ.get("KMG4_2", "0"))   # phase-C EXP4G units per 16
NMM = int(os.environ.get("KNMM", "512"))     # matmul moving width

_DVE_OPS = {}


def _ensure_dve_ops():
    """Register the SELU custom DVE ops (idempotent, process-wide)."""
    if _DVE_OPS:
        return _DVE_OPS
    import concourse.dve_ops as dve_ops
    from concourse.dve_spec import (
        C0, C1, C2, One, Spec, Src0, Src1, Zero,
        lower, minn, relu, sq,
    )
    from concourse.dve_spec import _has_src1 as has_src1
    from concourse.dve_uop import DveOpSpec

    def _pp(v, p):
        a = np.asarray(v, np.float32)
        if a.size == p:
            return a.reshape(p, 1)
        return np.float32(a.reshape(-1)[0]) if a.size else np.float32(0)

    # POLY: v = (1 + u + u^2/2)^2,  u = min(Src0*C0 + C1, 0)
    u = minn(Src0 * C0 + C1, Zero)
    poly_body = sq(One + u * (One + u * C2))

    def poly_ref(in0, in1, c0, c1, c2):
        x = in0.astype(np.float32)
        p = x.shape[0]
        sh = x.shape
        x2 = x.reshape(p, -1)
        uu = np.minimum(x2 * _pp(c0, p) + _pp(c1, p), 0.0)
        pp = 1.0 + uu * (1.0 + uu * np.float32(c2))
        return (pp * pp).astype(np.float32).reshape(sh)

    # COMB2: y = relu(Src0*C0 + C1) + Src1^16
    comb2_body = relu(Src0 * C0 + C1) + sq(sq(sq(sq(Src1))))

    def comb2_ref(in0, in1, c0, c1, c2):
        z = in0.astype(np.float32)
        p = z.shape[0]
        z2 = z.reshape(p, -1)
        v = in1.astype(np.float32).reshape(p, -1)
        r = np.maximum(z2 * _pp(c0, p) + _pp(c1, p), 0.0)
        return (r + v ** 16).astype(np.float32).reshape(in0.shape)

    # COMB4: y = relu(Src0*C0 + C1) + min(Src1, 1)   (Src1 = e^z f16;
    # min handles the z>0 branch, including inf from f16 exp overflow)
    comb4_body = relu(Src0 * C0 + C1) + minn(Src1, One)

    def comb4_ref(in0, in1, c0, c1, c2):
        z = in0.astype(np.float32)
        p = z.shape[0]
        z2 = z.reshape(p, -1)
        e = in1.astype(np.float32).reshape(p, -1)
        r = np.maximum(z2 * _pp(c0, p) + _pp(c1, p), 0.0)
        return (r + np.minimum(e, 1.0)).astype(np.float32).reshape(in0.shape)

    # COMB3: y = relu(Src0*C0 + C1) + Src1 * C2
    comb3_body = relu(Src0 * C0 + C1) + Src1 * C2

    def comb3_ref(in0, in1, c0, c1, c2):
        z = in0.astype(np.float32)
        p = z.shape[0]
        z2 = z.reshape(p, -1)
        e = in1.astype(np.float32).reshape(p, -1)
        r = np.maximum(z2 * _pp(c0, p) + _pp(c1, p), 0.0)
        return (r + e * np.float32(c2)).astype(np.float32).reshape(in0.shape)

    specs = [
        ("SELU_POLY_ANT", Spec(body=poly_body, reference=poly_ref)),
        ("SELU_COMB2_ANT", Spec(body=comb2_body, reference=comb2_ref)),
        ("SELU_COMB3_ANT", Spec(body=comb3_body, reference=comb3_ref)),
        ("SELU_COMB4_ANT", Spec(body=comb4_body, reference=comb4_ref)),
    ]
    for name, spec in specs:
        if name not in dve_ops._SUB_OPCODE_FOR_NAME:
            row = dve_ops._CUSTOM_DVE_ROW_BASE + len(dve_ops.OPS)
            assert row < 0x20
            dve_ops._SUB_OPCODE_FOR_NAME[name] = row
            sha = {}
            for ver in ("v3",):
                s = DveOpSpec(name=name, opcode=row, uops=lower(spec, ver=ver),
                              rd1_en=has_src1(spec))
                sha[ver] = s.sha(ver)
            op = dve_ops.DveOp(name, spec, subdim=False, uops_sha=sha)
            dve_ops.OPS.append(op)
            dve_ops.CUSTOM_DVE_SPECS[name] = spec
        _DVE_OPS[name] = next(o for o in dve_ops.OPS if o.name == name)
    return _DVE_OPS


def _host_prep(inputs):
    """All precomputation that depends only on small inputs (and x packing)."""
    x = np.asarray(inputs["x"], np.float32)
    pidx = np.asarray(inputs["parent_idx"], np.int64)
    W1 = np.asarray(inputs["W1"], np.float32)
    W2 = np.asarray(inputs["W2"], np.float32)
    W3 = np.asarray(inputs["W3"], np.float32)
    b3 = np.asarray(inputs["b3"], np.float32)
    B = x.shape[0]

    # per-core small-weight rows: for core r (nodes 32r..32r+32, channel
    # shard c in [0,256): node 32r+c//8, feature h=c%8), row d carries
    #   pidx[n, d] | W1[n, d, h] | LAM*AL*W2[n, d, h] | W3 cols (j-diag)
    # (LAM and the SELU alpha fold into W2/W3: the device computes
    #  y'' = relu(z)/AL + e^{min(z,0)}, so downstream weights carry LAM*AL)
    wsms = []
    cn = np.arange(CSH) // 8
    chh = np.arange(CSH) % 8
    j64 = np.arange(64)
    for r in range(NCORES):
        wsm = np.zeros((DD, SMCOLS), np.float16)
        nr = 32 * r + cn
        wsm[:, 0:CSH] = pidx[nr, :].T.astype(np.float16)
        for d in range(DD):
            wsm[d, SM_W1:SM_W1 + CSH] = W1[nr, d, chh]
            wsm[d, SM_W2:SM_W2 + CSH] = LAM * AL * W2[nr, d, chh]
            # w3 cols: (g-2r)*32 + j, value only for j < 16
            n3 = 32 * r + 16 * (j64 // 32) + (j64 % 32)
            valid = (j64 % 32) < 16
            wsm[d, SM_W3:SM_W3 + 64] = np.where(
                valid, LAM * AL * W3[np.minimum(n3, NSUB - 1), d, 0], 0.0)
        wsms.append(wsm)

    # b3 with the two -LAM*AL constant folds (see module docstring)
    b3p = b3[:, 0] - LAM * AL * W3[:, :, 0].sum(axis=1)
    b3t = np.zeros((128, 4), np.float32)
    for t in range(4):
        for j in range(4):
            b3t[32 * j:32 * j + 16, t] = b3p[(4 * t + j) * 16:(4 * t + j) * 16 + 16]

    # per-group [128, 16] layouts of gamma/beta
    P = np.arange(128)
    gof = P // 8
    hof = P % 8

    def grouped(v):
        out = np.zeros((128, G), np.float32)
        for g in range(G):
            out[:, g] = v[16 * g + gof, hof]
        return out

    gb = np.stack([grouped(np.asarray(inputs[k], np.float32))
                   for k in ("gamma1", "beta1", "gamma2", "beta2")], axis=-1)
    gbh = gb.reshape(128, 64).astype(np.float16)   # gamma/beta: f16-exact

    Bs = B // NCORES
    xTs = []
    for c in range(NCORES):
        xTs.append(np.ascontiguousarray(
            x[c * Bs:(c + 1) * Bs].T.astype(np.float16)))
    return xTs, wsms, gbh, b3t, B, Bs


def build_body(tc, outs, ins, B, Bs):
    """Emit the whole program into TileContext `tc`."""
    ops = _ensure_dve_ops()
    COMB4 = ops["SELU_COMB4_ANT"]
    nc = tc.nc
    AF = mybir.ActivationFunctionType
    NCH = Bs // CHUNK
    xT, wsm, gbh, b3t = ins["xT"], ins["wsm"], ins["gbh"], ins["b3t"]
    out_T = outs["out_T"]

    from contextlib import ExitStack
    stack = ExitStack()
    const = stack.enter_context(tc.tile_pool(name="const", bufs=1))
    dram = stack.enter_context(tc.tile_pool(name="dram", bufs=1, space="DRAM"))

    _sc_gram = nc.enter_named_scope("s_gram", False)[0]
    gbsb_h = const.tile([128, G, 4], F16, tag="gbh")
    nc.sync.dma_start(gbsb_h[:], gbh[:].rearrange("p (g f) -> p g f", f=4))
    gbsb = const.tile([128, G, 4], F32, tag="gb")
    nc.vector.tensor_copy(gbsb[:], gbsb_h[:])
    b3sb = const.tile([128, 4], F32, tag="b3")
    nc.sync.dma_start(b3sb[:], b3t[:])

    # resident xT chunks (feature-major) for L1 + gram lhs
    xk = []
    for k, (of, sz) in enumerate(KT):
        t = const.tile([sz, Bs], F16, tag=f"xk{k}", name=f"xk{k}")
        nc.sync.dma_start(t[:], xT[of:of + sz, :])
        xk.append(t)

    onesb = const.tile([128, 1], F16, tag="ones")
    nc.vector.memset(onesb[:], 1.0)

    # ---- build this core's channel-shard of the weight pack ---------------
    # wloc[p, kt*CSH + c] = sum_d [pidx==KT[kt].of+p] * W1-row, plus the
    # block-diagonal W2/W3 columns via affine masks.
    # all DD rows live on partition 0: partition_broadcast reads there only
    wsm_sb = const.tile([1, DD, SMCOLS], F16, tag="wsm")
    nc.gpsimd.dma_start(wsm_sb[:],
                        wsm[:].rearrange("(o d) c -> o d c", o=1))
    wloc = const.tile([128, WLOC], F16, tag="wloc")
    nc.vector.memset(wloc[:], 0.0)
    # A1 accumulates in f32 (duplicate parents sum >1 term), cast once
    a1f = const.tile([128, 3 * CSH], F32, tag="a1f")
    nc.vector.memset(a1f[:], 0.0)
    iota_i = const.tile([128, 1], mybir.dt.int32, tag="iota_i")
    nc.gpsimd.iota(iota_i[:], pattern=[[0, 1]], base=0, channel_multiplier=1)
    iota_h = const.tile([128, 1], F32, tag="iota_h")
    nc.vector.tensor_copy(iota_h[:], iota_i[:])
    iofs = const.tile([128, 3], F32, tag="iofs")
    for k, (of, sz) in enumerate(KT):
        nc.vector.tensor_scalar_add(iofs[:, k:k + 1].opt(), iota_h[:], float(of))
    with tc.tile_pool(name="wbp", bufs=2) as wbp:
        for d in range(DD):
            pb = wbp.tile([128, SMCOLS], F16, tag="pb")
            nc.gpsimd.partition_broadcast(pb[:], wsm_sb[0:1, d, :].opt())
            w1f = wbp.tile([128, CSH], F32, tag="w1f")
            nc.vector.tensor_copy(w1f[:], pb[:, SM_W1:SM_W1 + CSH].opt())
            for k, (of, sz) in enumerate(KT):
                msk = wbp.tile([128, CSH], F32, tag="msk")
                nc.vector.tensor_scalar(
                    msk[:], pb[:, 0:CSH].opt(), iofs[:, k:k + 1].opt(), None,
                    op0=mybir.AluOpType.is_equal)
                nc.vector.tensor_mul(msk[:], msk[:], w1f[:])
                nc.vector.tensor_add(a1f[:, CSH * k:CSH * (k + 1)].opt(),
                                     a1f[:, CSH * k:CSH * (k + 1)].opt(),
                                     msk[:])
            # W2 block-diag: keep where p == 8*j + d (per group, j of 16)
            m2 = wbp.tile([128, CSH], F16, tag="m2")
            nc.gpsimd.affine_select(
                m2[:].rearrange("p (g j h) -> p g j h", j=16, h=8),
                pb[:, SM_W2:SM_W2 + CSH].rearrange(
                    "p (g j h) -> p g j h", j=16, h=8),
                pattern=[[0, 2], [-8, 16], [0, 8]],
                compare_op=mybir.AluOpType.is_equal,
                fill=0.0, base=-d, channel_multiplier=1)
            nc.gpsimd.tensor_add(wloc[:, W2L:W2L + CSH].opt(),
                                 wloc[:, W2L:W2L + CSH].opt(), m2[:])
            # W3 block-diag: keep where p == 8*j + d (per group, j of 32)
            m3 = wbp.tile([128, 64], F16, tag="m3")
            nc.gpsimd.affine_select(
                m3[:].rearrange("p (g j) -> p g j", j=32),
                pb[:, SM_W3:SM_W3 + 64].rearrange("p (g j) -> p g j", j=32),
                pattern=[[0, 2], [-8, 32]],
                compare_op=mybir.AluOpType.is_equal,
                fill=0.0, base=-d, channel_multiplier=1)
            nc.gpsimd.tensor_add(wloc[:, W3L:WLOC].opt(),
                                 wloc[:, W3L:WLOC].opt(), m3[:])
    nc.scalar.activation(wloc[:, 0:3 * CSH].opt(), a1f[:], AF.Copy)

    # ---- weight AllGather (gpsimd queue; overlaps the gram) ---------------
    # Re-pack the built [128, WLOC] shard into 16 rows via a DRAM bounce so
    # the collective runs in the proven [16, WAGC] -> [128, WAGC] shape,
    # then un-pack the gathered result to rank-major [1024, WLOC].
    wtmp1 = dram.tile([128, WLOC], F16)
    wagin = dram.tile([16, WAGC], F16)
    wagout = dram.tile([128, WAGC], F16, addr_space="Shared")
    wtmp2 = dram.tile([NCORES * 128, WLOC], F16)
    nc.gpsimd.dma_start(wtmp1[:], wloc[:])
    nc.gpsimd.dma_start(
        wagin[:].rearrange("q (blk c) -> q blk c", blk=NCORES),
        wtmp1[:].rearrange("(blk q) c -> q blk c", q=16))
    nc.gpsimd.collective_compute(
        "AllGather", mybir.AluOpType.bypass,
        replica_groups=[list(range(NCORES))],
        ins=[wagin[:].opt()], outs=[wagout[:].opt()])
    for r in range(NCORES):
        nc.gpsimd.dma_start(
            wtmp2[128 * r:128 * (r + 1), :]
            .rearrange("(blk q) c -> blk q c", q=16),
            wagout[16 * r:16 * (r + 1), :]
            .rearrange("q (blk c) -> blk q c", blk=NCORES))

    # ---- phase 1: C = x^T x gram via on-device XBAR transposes ------------
    # column NTOT of C carries the batch column-sums (ones-matmul), giving
    # the mean row for BN1 after the projection.
    csb = [const.tile([sz, NTOT + 1], F16, tag=f"c_{m}", name=f"c_{m}")
           for m, (of, sz) in enumerate(KT)]
    gram_ctx = ExitStack()
    xbp_pool = gram_ctx.enter_context(tc.tile_pool(name="xbp", bufs=3))
    cps_pool = gram_ctx.enter_context(
        tc.tile_pool(name="cps", bufs=1, space="PSUM"))
    cps = [cps_pool.tile([sz, NTOT + 1], F32, tag=f"cps{m}", name=f"cps{m}")
           for m, (of, sz) in enumerate(KT)]
    TRB = 1024                      # b-columns per transpose tile
    ntr = Bs // TRB
    for i in range(ntr):
        xb = xbp_pool.tile([128, TRB // 128, NTOT], F16, tag="xb")
        nc.sync.dma_start_transpose(xb[:], xT[:, TRB * i:TRB * (i + 1)])
        for b in range(TRB // 128):
            first = (i == 0 and b == 0)
            last = (i == ntr - 1 and b == TRB // 128 - 1)
            for m, (of, sz) in enumerate(KT):
                nc.tensor.matmul(cps[m][:, 0:NTOT],
                                 xb[:, b, of:of + sz].opt(),
                                 xb[:, b, :].opt(),
                                 start=first, stop=last)
                nc.tensor.matmul(cps[m][:, NTOT:NTOT + 1],
                                 xb[:, b, of:of + sz].opt(),
                                 onesb[:],
                                 start=first, stop=last)

    # local C / B -> f16 in SBUF; the cross-core reduction happens later
    # on the tiny projected stats (e2row/meanrow), not on C itself.
    for m, (of, sz) in enumerate(KT):
        nc.scalar.activation(csb[m][:], cps[m][:], AF.Identity, scale=1.0 / B)
    gram_ctx.close()

    # resident weight tiles assembled from the gathered rank shards
    a1sb = [const.tile([sz, CH], F16, tag=f"a1_{k}", name=f"a1_{k}")
            for k, (of, sz) in enumerate(KT)]
    w2all = const.tile([128, G * 128], F16, tag="w2all")
    w3all = const.tile([128, G * 32], F16, tag="w3all")
    for r in range(NCORES):
        for k, (of, sz) in enumerate(KT):
            nc.gpsimd.dma_start(
                a1sb[k][:, CSH * r:CSH * (r + 1)],
                wtmp2[128 * r:128 * r + sz, CSH * k:CSH * (k + 1)])
        nc.gpsimd.dma_start(w2all[:, CSH * r:CSH * (r + 1)],
                            wtmp2[128 * r:128 * (r + 1), W2L:W2L + CSH])
        nc.gpsimd.dma_start(w3all[:, 64 * r:64 * (r + 1)],
                            wtmp2[128 * r:128 * (r + 1), W3L:WLOC])
    w2sb = [w2all[:, 128 * g:128 * (g + 1)].opt() for g in range(G)]
    w3sb = [w3all[:, 32 * g:32 * (g + 1)].opt() for g in range(G)]

    nc.leave_named_scope("s_gram", _sc_gram, False)
    _sc_bn1 = nc.enter_named_scope("s_bn1", False)[0]

    # ---- phase 3: BN1 parameters from C ----------------------------------
    # T = (C/B) @ A1 ; E[h1^2] = colsum(A1 * T[0:320]) ; mean1 = C-col @ A1
    e2sb = const.tile([1, CH], F32, tag="e2row")
    meanrow = const.tile([1, CH], F32, tag="meanrow")
    with (tc.tile_pool(name="p3w", bufs=2) as p3w,
          tc.tile_pool(name="p3ps", bufs=1, space="PSUM") as p3ps):
        e2ps = p3ps.tile([1, CH], F32, tag="e2ps")
        mps = p3ps.tile([1, CH], F32, tag="tps")
        for nn in range(CH // 512):
            for k, (kof, ksz) in enumerate(KT):
                nc.tensor.matmul(
                    mps[:, 512 * nn:512 * (nn + 1)],
                    csb[k][:ksz, NTOT:NTOT + 1],
                    a1sb[k][:, 512 * nn:512 * (nn + 1)],
                    start=(k == 0), stop=(k == 2))
        nc.scalar.activation(meanrow[:], mps[:], AF.Copy)
        for m, (of, sz) in enumerate(KT):
            tps = p3ps.tile([sz, CH], F32, tag="tps")
            for nn in range(CH // 512):
                for k, (kof, ksz) in enumerate(KT):
                    nc.tensor.matmul(tps[:, 512 * nn:512 * (nn + 1)],
                                     csb[k][:ksz, of:of + sz],
                                     a1sb[k][:, 512 * nn:512 * (nn + 1)],
                                     start=(k == 0), stop=(k == 2))
            tf = p3w.tile([sz, CH], F16, tag="tf")
            nc.scalar.activation(tf[:], tps[:], AF.Copy)
            prod = p3w.tile([sz, CH], F16, tag="prod")
            nc.vector.tensor_mul(prod[:], a1sb[m][:], tf[:])
            for nn in range(CH // 512):
                nc.tensor.matmul(e2ps[:, 512 * nn:512 * (nn + 1)],
                                 onesb[:sz, :],
                                 prod[:, 512 * nn:512 * (nn + 1)],
                                 start=(m == 0), stop=(m == 2))
        nc.vector.tensor_copy(e2sb[:], e2ps[:])

    # AllReduce the per-core projected stats (e2row, meanrow: 16KB) and
    # reshape the [1, 2048] rows -> [128, 16] group layout via a DRAM bounce
    e2g = const.tile([128, G], F32, tag="e2g")
    m1g = const.tile([128, G], F32, tag="m1g")
    rowbounce = dram.tile([2, CH], F32)
    rbout = dram.tile([2, CH], F32)
    nc.gpsimd.dma_start(rowbounce[0:1, :], e2sb[:])
    nc.gpsimd.dma_start(rowbounce[1:2, :], meanrow[:])
    nc.gpsimd.collective_compute(
        "AllReduce", mybir.AluOpType.add,
        replica_groups=[list(range(NCORES))],
        ins=[rowbounce[:].opt()], outs=[rbout[:].opt()])
    nc.sync.dma_start(e2g[:],
                      rbout[0:1, :].rearrange("o (g p) -> (o p) g", p=128))
    nc.sync.dma_start(m1g[:],
                      rbout[1:2, :].rearrange("o (g p) -> (o p) g", p=128))

    def bn_params(mean_t, e2_t, gamma_ap, beta_ap, pool):
        """-> (sA, tA, s, t) [128, G] f32 tiles."""
        var = pool.tile([128, G], F32, tag="var")
        nc.vector.tensor_mul(var[:], mean_t[:], mean_t[:])
        nc.vector.tensor_sub(var[:], e2_t[:], var[:])
        nc.vector.tensor_scalar_add(var[:], var[:], EPS)
        sq_ = pool.tile([128, G], F32, tag="sqv")
        nc.scalar.activation(sq_[:], var[:], AF.Sqrt)
        r0 = pool.tile([128, G], F32, tag="r0")
        nc.vector.reciprocal(r0[:], sq_[:])
        # one Newton step for rsqrt: r = r0*(1.5 - 0.5*var*r0^2)
        t1_ = pool.tile([128, G], F32, tag="nt1")
        nc.vector.tensor_mul(t1_[:], r0[:], r0[:])
        nc.vector.tensor_mul(t1_[:], var[:], t1_[:])
        nc.vector.tensor_scalar(t1_[:], t1_[:], -0.5, 1.5,
                                op0=mybir.AluOpType.mult, op1=mybir.AluOpType.add)
        rs = pool.tile([128, G], F32, tag="rs")
        nc.vector.tensor_mul(rs[:], r0[:], t1_[:])
        s = pool.tile([128, G], F32, tag="s")
        nc.vector.tensor_mul(s[:], gamma_ap, rs[:])
        t = pool.tile([128, G], F32, tag="t")
        nc.vector.tensor_mul(t[:], s[:], mean_t[:])
        nc.vector.tensor_sub(t[:], beta_ap, t[:])
        sA = pool.tile([128, G], F32, tag="sA")
        nc.vector.tensor_scalar_mul(sA[:], s[:], 1.0 / AL)
        tA = pool.tile([128, G], F32, tag="tA")
        nc.vector.tensor_scalar_mul(tA[:], t[:], 1.0 / AL)
        return sA, tA, s, t

    nc.leave_named_scope("s_bn1", _sc_bn1, False)
    _sc_bnp = nc.enter_named_scope("s_bnp1", False)[0]
    s1A, t1A, s1, t1 = bn_params(
        m1g, e2g, gbsb[:, :, 0].opt(), gbsb[:, :, 1].opt(), const)
    nc.leave_named_scope("s_bnp1", _sc_bnp, False)
    if "dbg" in outs:
        for i, tt_ in enumerate((e2g, m1g, s1A, t1A)):
            nc.gpsimd.dma_start(outs["dbg"][:, 16 * i:16 * (i + 1)], tt_[:])
        nc.gpsimd.dma_start(outs["dbgc"][:], csb[0][:])

    def selu_unit(u, wk, q_ap, y_ap, prm, me4, mg4):
        """Emit one [128, CHUNK] SELU: y'' = relu(z)/AL + e^{min(z,0)},
        z = s*q + t, via e^{min(z,0)} = min(e^z, 1).
        q_ap: f32 PSUM input; y_ap: f16 SBUF output slice.
        prm = (sA, tA, s, t) [128, 1] slices of the unit group.
        Paths per u%16: [0, mg4) EXP4G (2 scalar + gpsimd min&add),
        [mg4, mg4+me4) EXP4 (2 scalar + vector min + gpsimd add),
        rest COMB4 (1 scalar exp + 1 vector combine).
        """
        sA, tA, s, t = prm
        r = u % 16
        ex = wk.tile([128, CHUNK], F16, tag="ex")
        nc.scalar.activation(ex[:], q_ap, AF.Exp, bias=t, scale=s)
        if r < mg4 + me4:
            r1 = wk.tile([128, CHUNK], F16, tag="r1")
            nc.scalar.activation(r1[:], q_ap, AF.Relu, bias=tA, scale=sA)
            exm = wk.tile([128, CHUNK], F16, tag="exm")
            if r < mg4:
                nc.gpsimd.tensor_scalar_min(exm[:], ex[:], 1.0)
            else:
                nc.vector.tensor_scalar_min(exm[:], ex[:], 1.0)
            nc.gpsimd.tensor_add(y_ap, exm[:], r1[:])
        else:
            nc.vector._custom_dve(COMB4, out=y_ap, in0=q_ap, in1=ex[:],
                                  s0=sA, s1=tA)

    # ---- phase B: h1 -> selu -> y1 spill + h2 stats ----------------------
    _sc_phB = nc.enter_named_scope("s_phB", False)[0]
    spill = dram.tile([G, 128, Bs], F16)
    bnstash = const.tile([128, G, 12 * NCH], F32, tag="bnstash")
    mv = const.tile([128, G, 2], F32, tag="mv")
    with (tc.tile_pool(name="y1p", bufs=2) as y1p,
          tc.tile_pool(name="qps", bufs=3, space="PSUM") as qps_pool,
          tc.tile_pool(name="hps", bufs=2, space="PSUM") as hps_pool,
          tc.tile_pool(name="wk", bufs=4) as wk):
        for g in range(G):
            y1g = y1p.tile([128, Bs], F16, tag="y1g")
            prm1 = tuple(p[:, g:g + 1].opt() for p in (s1A, t1A, s1, t1))
            for c in range(NCH):
                u = g * NCH + c
                q = qps_pool.tile([128, CHUNK], F32, tag="q")
                for h in range(CHUNK // NMM):
                    for k, (kof, ksz) in enumerate(KT):
                        nc.tensor.matmul(
                            q[:, NMM * h:NMM * (h + 1)],
                            a1sb[k][:, 128 * g:128 * (g + 1)].opt(),
                            xk[k][:, c * CHUNK + NMM * h:
                                  c * CHUNK + NMM * (h + 1)].opt(),
                            start=(k == 0), stop=(k == 2))
                ysl = y1g[:, c * CHUNK:(c + 1) * CHUNK].opt()
                selu_unit(u, wk, q[:], ysl, prm1, ME4_1, MG4_1)
                # h2 in [128, 512] half-tiles (1 PSUM bank each) so qps
                # can triple-buffer; each half feeds bn_stats immediately
                for h in range(CHUNK // 512):
                    h2 = hps_pool.tile([128, 512], F32, tag="h2")
                    nc.tensor.matmul(h2[:], w2sb[g],
                                     y1g[:, c * CHUNK + 512 * h:
                                         c * CHUNK + 512 * (h + 1)],
                                     start=True, stop=True)
                    nc.vector.bn_stats(
                        bnstash[:, g, 12 * c + 6 * h:12 * c + 6 * (h + 1)].opt(),
                        h2[:])
            nc.sync.dma_start(spill[g], y1g[:])
            nc.vector.bn_aggr(mv[:, g, :].opt(), bnstash[:, g, :].opt())

    # ---- stats2 AllReduce ------------------------------------------------
    # convert to (mean, meansq) * (1/NCORES) so the AllReduce sums to global
    nc.leave_named_scope("s_phB", _sc_phB, False)
    _sc_ar2 = nc.enter_named_scope("s_ar2", False)[0]
    ssq = const.tile([128, G, 2], F32, tag="ssq")
    nc.vector.tensor_mul(ssq[:, :, 0].opt(), mv[:, :, 0].opt(), mv[:, :, 0].opt())
    nc.vector.tensor_add(ssq[:, :, 1].opt(), mv[:, :, 1].opt(), ssq[:, :, 0].opt())
    nc.vector.tensor_scalar_mul(ssq[:, :, 1].opt(), ssq[:, :, 1].opt(), 1.0 / NCORES)
    nc.vector.tensor_scalar_mul(ssq[:, :, 0].opt(), mv[:, :, 0].opt(), 1.0 / NCORES)
    stin = dram.tile([128, G, 2], F32)
    stout = dram.tile([128, G, 2], F32)
    nc.sync.dma_start(stin[:], ssq[:])
    nc.gpsimd.collective_compute(
        "AllReduce", mybir.AluOpType.add, replica_groups=[list(range(NCORES))],
        ins=[stin[:].opt()], outs=[stout[:].opt()])
    gst = const.tile([128, G, 2], F32, tag="gst")
    nc.sync.dma_start(gst[:], stout[:])
    m2g = const.tile([128, G], F32, tag="m2g")
    e2g2 = const.tile([128, G], F32, tag="e2g2")
    nc.vector.tensor_copy(m2g[:], gst[:, :, 0].opt())
    nc.vector.tensor_copy(e2g2[:], gst[:, :, 1].opt())
    s2A, t2A, s2, t2 = bn_params(
        m2g, e2g2, gbsb[:, :, 2].opt(), gbsb[:, :, 3].opt(), const)
    if "dbg" in outs:
        for i, tt_ in enumerate((e2g2, m2g, s2A, t2A)):
            nc.gpsimd.dma_start(outs["dbg"][:, 64 + 16 * i:64 + 16 * (i + 1)],
                                tt_[:])

    # ---- phase C: reload y1 -> h2 -> selu -> out -------------------------
    nc.leave_named_scope("s_ar2", _sc_ar2, False)
    _sc_phC = nc.enter_named_scope("s_phC", False)[0]
    with (tc.tile_pool(name="y4p", bufs=4) as y4p,
          tc.tile_pool(name="hps2", bufs=3, space="PSUM") as hps2,
          tc.tile_pool(name="ops2", bufs=1, space="PSUM") as ops_pool,
          tc.tile_pool(name="wk2", bufs=4) as wk2):
        for ti in range(4):
            for c in range(NCH):
                y4 = y4p.tile([128, 4, CHUNK], F16, tag="y4")
                nc.sync.dma_start(
                    y4[:],
                    spill[4 * ti:4 * ti + 4, :, c * CHUNK:(c + 1) * CHUNK]
                    .rearrange("j p n -> p j n"))
                op = ops_pool.tile([128, CHUNK], F32, tag="op")
                for j in range(4):
                    g = 4 * ti + j
                    u = (ti * NCH + c) * 4 + j
                    prm2 = tuple(p[:, g:g + 1].opt()
                                 for p in (s2A, t2A, s2, t2))
                    h2 = hps2.tile([128, CHUNK], F32, tag="h2c")
                    for h in range(CHUNK // NMM):
                        nc.tensor.matmul(
                            h2[:, NMM * h:NMM * (h + 1)], w2sb[g],
                            y4[:, j, NMM * h:NMM * (h + 1)].opt(),
                            start=True, stop=True)
                    y2 = wk2.tile([128, CHUNK], F16, tag="y2")
                    selu_unit(u, wk2, h2[:], y2[:].opt(), prm2, ME4_2, MG4_2)
                    if "dbgy2" in outs and ti == 0 and c == 0 and j == 0:
                        nc.gpsimd.dma_start(outs["dbgy2"][0:128, :], y2[:])
                    for h in range(CHUNK // 512):
                        nc.tensor.matmul(
                            op[32 * j:32 * j + 32, 512 * h:512 * (h + 1)],
                            w3sb[g], y2[:, 512 * h:512 * (h + 1)],
                            start=True, stop=True, tile_position=(0, 32 * j))
                osb = wk2.tile([128, CHUNK], F16, tag="osb")
                nc.vector.tensor_scalar_add(osb[:], op[:],
                                            b3sb[:, ti:ti + 1].opt())
                for j in range(4):
                    nc.sync.dma_start(
                        out_T[64 * ti + 16 * j:64 * ti + 16 * (j + 1),
                              c * CHUNK:(c + 1) * CHUNK],
                        osb[32 * j:32 * j + 16, :])
    nc.leave_named_scope("s_phC", _sc_phC, False)
    stack.close()


_PROGRAM_CACHE = {}


def _build_program(B, Bs):
    key = (B, Bs, ME4_1, ME4_2, MG4_1, MG4_2, NMM, os.environ.get("KDBG", ""))
    if key in _PROGRAM_CACHE:
        return _PROGRAM_CACHE[key]
    nc = bacc.Bacc("TRN2", target_bir_lowering=False, debug=False,
                   enable_asserts=False, num_devices=NCORES)
    ins = {
        "xT": nc.dram_tensor("xT", [NTOT, Bs], F16, kind="ExternalInput").ap(),
        "wsm": nc.dram_tensor("wsm", [DD, SMCOLS], F16,
                              kind="ExternalInput").ap(),
        "gbh": nc.dram_tensor("gbh", [128, 64], F16,
                              kind="ExternalInput").ap(),
        "b3t": nc.dram_tensor("b3t", [128, 4], F32,
                              kind="ExternalInput").ap(),
    }
    outs = {"out_T": nc.dram_tensor("out_T", [NSUB, Bs], F16,
                                    kind="ExternalOutput").ap()}
    if os.environ.get("KDBG"):
        outs["dbg"] = nc.dram_tensor("dbg", [128, 16 * 8], F32,
                                     kind="ExternalOutput").ap()
        outs["dbgc"] = nc.dram_tensor("dbgc", [128, NTOT], F16,
                                      kind="ExternalOutput").ap()
        outs["dbgy2"] = nc.dram_tensor("dbgy2", [256, 1024], F16,
                                       kind="ExternalOutput").ap()
    with tile.TileContext(nc) as tc:
        build_body(tc, outs, ins, B, Bs)
    nc.finalize()
    _PROGRAM_CACHE[key] = nc
    return nc


def kernel(**inputs) -> np.ndarray:
    xTs, wsms, gbh, b3t, B, Bs = _host_prep(inputs)
    nc = _build_program(B, Bs)
    in_maps = []
    for c in range(NCORES):
        in_maps.append({"xT": xTs[c], "wsm": wsms[c],
                        "gbh": gbh, "b3t": b3t})
    res = run_bass_kernel_spmd(nc, in_maps, core_ids=list(range(NCORES)))
    kernel.last_results = res
    out = np.concatenate(
        [np.asarray(r["out_T"]).T.astype(np.float32) for r in res.results], axis=0)
    # sigmoid of the sink node runs on host
    out[:, -1] = 1.0 / (1.0 + np.exp(-out[:, -1]))
    return out
